# revision 2
# baseline (speedup 1.0000x reference)
"""Trainium2 Bass kernel for DoubleBinaryLinear:
    y = ((x * s0) @ B.T * s2) @ A.T * s4 + bias
with x [4, 2048, 4096] fp32 and binary (+-1) B, A [4096, 4096].

Strategy
--------
Data-parallel over tokens: the 8192 tokens are split 1024 per NeuronCore
(8 cores), each core runs the full two-layer pipeline on its token slice.
No collectives. Weights are replicated (B.T / A.T, cast to bf16 — exact
for +-1 entries).

On-chip dataflow keeps activations transposed ([feature_partition,
token_free]) so every diagonal scaling is a per-partition tensor_scalar op
and matmul contractions run over the partition axis:

  stage 0: z  = bf16(xT * s0)
  stage 1: h1 = B @ z       -- PSUM fp32 accumulation
  stage 2: h1s = bf16(h1 * s2)
  stage 3: h2 = A @ h1s     -- PSUM fp32 accumulation
  stage 4: yT = h2 * s4 + bias

Single-pass bf16: activations are rounded to bf16 once per stage while
the +-1 weights are exact in bf16, so each matmul runs one 1-cycle/row
bf16 pass.  The only error is the two bf16 roundings of the activations
(~2^-8 relative each); measured end-to-end error ~2.5e-3 of the output
scale — well under the 2e-2 gate — at 2x the speed of the hi/lo
two-pass scheme (4096 matmuls/core instead of 8192).

TensorE bound: 4096 matmuls x 216 ns = 885 us/core.
"""

import os

import numpy as np
import ml_dtypes

import concourse.bacc as bacc
import concourse.mybir as mybir
from concourse import tile
from concourse import bass_utils

P = 128
F32 = mybir.dt.float32
BF16 = mybir.dt.bfloat16

IN_D = 4096
MID_D = 4096
OUT_D = 4096
BATCH = 4
SEQ = 2048
N_CORES = 8
T_CORE = (BATCH * SEQ) // N_CORES   # 1024 tokens per core
TC = 512                            # token chunk = matmul moving free dim
MG = 4                              # m-tiles per PSUM group


def _build_nc():
    nI, nM, nO, nC = IN_D // P, MID_D // P, OUT_D // P, T_CORE // TC

    nc = bacc.Bacc(None, target_bir_lowering=False)
    xT = nc.dram_tensor("xT", [IN_D, T_CORE], F32, kind="ExternalInput")
    BTd = nc.dram_tensor("BT", [IN_D, MID_D], BF16, kind="ExternalInput")
    ATd = nc.dram_tensor("AT", [MID_D, OUT_D], BF16, kind="ExternalInput")
    nSC = (IN_D + MID_D + 2 * OUT_D) // P
    scd = nc.dram_tensor("sc", [P, nSC], F32, kind="ExternalInput")
    yT = nc.dram_tensor("yT", [OUT_D, T_CORE], F32, kind="ExternalOutput")

    mult = mybir.AluOpType.mult
    add = mybir.AluOpType.add

    with tile.TileContext(nc) as tc:
        with (
            tc.tile_pool(name="consts", bufs=1) as cpool,
            tc.tile_pool(name="zbuf", bufs=1) as zpool,
            tc.tile_pool(name="h1buf", bufs=1) as hpool,
            tc.tile_pool(name="xin", bufs=4) as xpool,
            tc.tile_pool(name="wts", bufs=6) as wpool,
            tc.tile_pool(name="yout", bufs=3) as ypool,
            tc.tile_pool(name="psum", bufs=8, space="PSUM") as pspool,
        ):
            sc_t = cpool.tile([P, nSC], F32, tag="sc")
            nc.sync.dma_start(sc_t[:], scd[:, :])
            s0_t = sc_t[:, 0:nI]
            s2_t = sc_t[:, nI:nI + nM]
            s4_t = sc_t[:, nI + nM:nI + nM + nO]
            bi_t = sc_t[:, nI + nM + nO:nSC]

            for c in range(nC):
                t0 = c * TC
                # stage 0: load x, scale by s0, round to bf16
                z = [zpool.tile([P, TC], BF16, tag=f"z{i}", name=f"z{i}")
                     for i in range(nI)]
                for i in range(nI):
                    xt = xpool.tile([P, TC], F32, tag="xt")
                    nc.scalar.dma_start(xt[:], xT[i * P:(i + 1) * P, t0:t0 + TC])
                    if c == 0 and i < 2:
                        nc.vector.tensor_scalar_mul(
                            z[i][:], xt[:], s0_t[:, i:i + 1])
                    else:
                        nc.scalar.activation(z[i][:], xt[:],
                                             mybir.ActivationFunctionType.Copy,
                                             scale=s0_t[:, i:i + 1])

                # stage 1: h1 = B @ z; stage 2: h1s = bf16(h1 * s2)
                h1 = [hpool.tile([P, TC], BF16, tag=f"h1{m}", name=f"h1{m}")
                      for m in range(nM)]
                for mg in range(nM // MG):
                    pss = [pspool.tile([P, TC], F32, tag="ps", name="ps")
                           for _ in range(MG)]
                    for i in range(nI):
                        wt = wpool.tile([P, MG * P], BF16, tag="wb")
                        nc.sync.dma_start(
                            wt[:], BTd[i * P:(i + 1) * P,
                                       mg * MG * P:(mg + 1) * MG * P])
                        for m_ in range(MG):
                            lhsT = wt[:, m_ * P:(m_ + 1) * P]
                            nc.tensor.matmul(pss[m_][:], lhsT, z[i][:],
                                             start=(i == 0), stop=(i == nI - 1))
                    for m_ in range(MG):
                        m = mg * MG + m_
                        nc.scalar.activation(
                            h1[m][:], pss[m_][:],
                            mybir.ActivationFunctionType.Copy,
                            scale=s2_t[:, m:m + 1])

                # stage 3: h2 = A @ h1s; stage 4: y = h2*s4 + bias
                for og in range(nO // MG):
                    pso = [pspool.tile([P, TC], F32, tag="ps", name="ps")
                           for _ in range(MG)]
                    for m in range(nM):
                        wt2 = wpool.tile([P, MG * P], BF16, tag="wa")
                        nc.sync.dma_start(
                            wt2[:], ATd[m * P:(m + 1) * P,
                                        og * MG * P:(og + 1) * MG * P])
                        for o_ in range(MG):
                            lhsT = wt2[:, o_ * P:(o_ + 1) * P]
                            nc.tensor.matmul(pso[o_][:], lhsT, h1[m][:],
                                             start=(m == 0), stop=(m == nM - 1))
                    for o_ in range(MG):
                        o = og * MG + o_
                        yt = ypool.tile([P, TC], F32, tag="yt")
                        nc.vector.tensor_scalar(
                            yt[:], pso[o_][:], s4_t[:, o:o + 1], bi_t[:, o:o + 1],
                            mult, add)
                        nc.scalar.dma_start(yT[o * P:(o + 1) * P, t0:t0 + TC], yt[:])

    nc.compile()
    return nc


_NC_CACHE = None


def _get_nc():
    global _NC_CACHE
    if _NC_CACHE is None:
        _NC_CACHE = _build_nc()
    return _NC_CACHE


def _col_major(v):
    return np.ascontiguousarray(
        np.asarray(v, dtype=np.float32).reshape(-1, P).T)


def make_in_maps(x, scaling0, B, scaling2, A, scaling4, bias):
    x = np.asarray(x, dtype=np.float32)
    xf = np.ascontiguousarray(x.reshape(BATCH * SEQ, IN_D))
    BT = np.ascontiguousarray(np.asarray(B, dtype=np.float32).T
                              ).astype(ml_dtypes.bfloat16)
    AT = np.ascontiguousarray(np.asarray(A, dtype=np.float32).T
                              ).astype(ml_dtypes.bfloat16)
    sc = np.ascontiguousarray(np.concatenate(
        [_col_major(v) for v in (scaling0, scaling2, scaling4, bias)], axis=1))

    in_maps = []
    for c in range(N_CORES):
        xs = xf[c * T_CORE:(c + 1) * T_CORE]
        in_maps.append({
            "xT": np.ascontiguousarray(xs.T),
            "BT": BT, "AT": AT, "sc": sc,
        })
    return in_maps


def kernel(x, scaling0, B, scaling2, A, scaling4, bias):
    # The profile hook isn't available in every environment; force the
    # plain execution path.
    os.environ.setdefault("BASS_NEVER_TRACE", "1")

    in_maps = make_in_maps(x, scaling0, B, scaling2, A, scaling4, bias)
    nc = _get_nc()
    res = bass_utils.run_bass_kernel_spmd(
        nc, in_maps, core_ids=list(range(N_CORES)))

    y = np.empty((BATCH * SEQ, OUT_D), dtype=np.float32)
    for c in range(N_CORES):
        y[c * T_CORE:(c + 1) * T_CORE] = res.results[c]["yT"].T
    return y.reshape(BATCH, SEQ, OUT_D)


# revision 5
# speedup vs baseline: 1.1294x; 1.1294x over previous
"""Trainium2 Bass kernel for DoubleBinaryLinear:
    y = ((x * s0) @ B.T * s2) @ A.T * s4 + bias
with x [4, 2048, 4096] fp32 and binary (+-1) B, A [4096, 4096].

Strategy
--------
Data-parallel over tokens: the 8192 tokens are split 1024 per NeuronCore
(8 cores), each core runs the full two-layer pipeline on its token slice.
No collectives. Weights are replicated (B.T / A.T, cast to bf16 — exact
for +-1 entries).

On-chip dataflow keeps activations transposed ([feature_partition,
token_free]) so every diagonal scaling is a per-partition tensor_scalar op
and matmul contractions run over the partition axis:

  stage 0: z  = bf16(xT * s0)
  stage 1: h1 = B @ z       -- PSUM fp32 accumulation
  stage 2: h1s = bf16(h1 * s2)
  stage 3: h2 = A @ h1s     -- PSUM fp32 accumulation
  stage 4: yT = h2 * s4 + bias

Single-pass bf16: activations are rounded to bf16 once per stage while
the +-1 weights are exact in bf16, so each matmul runs one 1-cycle/row
bf16 pass.  The only error is the two bf16 roundings of the activations
(~2^-8 relative each); measured end-to-end error ~2.5e-3 of the output
scale — well under the 2e-2 gate — at 2x the speed of the hi/lo
two-pass scheme (4096 matmuls/core instead of 8192).

TensorE bound: 4096 matmuls x 216 ns = 885 us/core.
"""

import os

import numpy as np
import ml_dtypes

import concourse.bacc as bacc
import concourse.mybir as mybir
from concourse import tile
from concourse import bass_utils

P = 128
F32 = mybir.dt.float32
BF16 = mybir.dt.bfloat16

IN_D = 4096
MID_D = 4096
OUT_D = 4096
BATCH = 4
SEQ = 2048
N_CORES = 8
T_CORE = (BATCH * SEQ) // N_CORES   # 1024 tokens per core
TC = 512                            # token chunk = matmul moving free dim
MG = 4                              # m-tiles per PSUM group


def _build_nc():
    nI, nM, nO, nC = IN_D // P, MID_D // P, OUT_D // P, T_CORE // TC

    nc = bacc.Bacc(None, target_bir_lowering=False)
    xT = nc.dram_tensor("xT", [IN_D, T_CORE], F32, kind="ExternalInput")
    BTd = nc.dram_tensor("BT", [IN_D, MID_D], BF16, kind="ExternalInput")
    ATd = nc.dram_tensor("AT", [MID_D, OUT_D], BF16, kind="ExternalInput")
    nSC = (IN_D + MID_D + 2 * OUT_D) // P
    scd = nc.dram_tensor("sc", [P, nSC], F32, kind="ExternalInput")
    yT = nc.dram_tensor("yT", [OUT_D, T_CORE], F32, kind="ExternalOutput")

    mult = mybir.AluOpType.mult
    add = mybir.AluOpType.add

    with tile.TileContext(nc) as tc:
        with (
            tc.tile_pool(name="consts", bufs=1) as cpool,
            tc.tile_pool(name="zbuf", bufs=1) as zpool,
            tc.tile_pool(name="h1buf", bufs=1) as hpool,
            tc.tile_pool(name="xin", bufs=10) as xpool,
            tc.tile_pool(name="wts", bufs=16) as wpool,
            tc.tile_pool(name="yout", bufs=3) as ypool,
            tc.tile_pool(name="psum", bufs=8, space="PSUM") as pspool,
        ):
            sc_t = cpool.tile([P, nSC], F32, tag="sc")
            nc.sync.dma_start(sc_t[:], scd[:, :])
            s0_t = sc_t[:, 0:nI]
            s2_t = sc_t[:, nI:nI + nM]
            s4_t = sc_t[:, nI + nM:nI + nM + nO]
            bi_t = sc_t[:, nI + nM + nO:nSC]

            for c in range(nC):
                t0 = c * TC
                # stage 0: load x, scale by s0, round to bf16
                z = [zpool.tile([P, TC], BF16, tag=f"z{i}", name=f"z{i}")
                     for i in range(nI)]
                for i in range(nI):
                    xt = xpool.tile([P, TC], F32, tag="xt")
                    nc.scalar.dma_start(xt[:], xT[i * P:(i + 1) * P, t0:t0 + TC])
                    # alternate engines so z production (400 ns/tile
                    # effective) outruns matmul consumption (853 ns/tile)
                    if (c == 0 and i < 2) or i % 2 == 1:
                        nc.vector.tensor_scalar_mul(
                            z[i][:], xt[:], s0_t[:, i:i + 1])
                    else:
                        nc.scalar.activation(z[i][:], xt[:],
                                             mybir.ActivationFunctionType.Copy,
                                             scale=s0_t[:, i:i + 1])

                # stage 1: h1 = B @ z; stage 2: h1s = bf16(h1 * s2)
                h1 = [hpool.tile([P, TC], BF16, tag=f"h1{m}", name=f"h1{m}")
                      for m in range(nM)]
                for mg in range(nM // MG):
                    pss = [pspool.tile([P, TC], F32, tag="ps", name="ps")
                           for _ in range(MG)]
                    for i in range(nI):
                        wt = wpool.tile([P, MG * P], BF16, tag="wb")
                        nc.sync.dma_start(
                            wt[:], BTd[i * P:(i + 1) * P,
                                       mg * MG * P:(mg + 1) * MG * P])
                        for m_ in range(MG):
                            lhsT = wt[:, m_ * P:(m_ + 1) * P]
                            nc.tensor.matmul(pss[m_][:], lhsT, z[i][:],
                                             start=(i == 0), stop=(i == nI - 1))
                    for m_ in range(MG):
                        m = mg * MG + m_
                        nc.scalar.activation(
                            h1[m][:], pss[m_][:],
                            mybir.ActivationFunctionType.Copy,
                            scale=s2_t[:, m:m + 1])

                # stage 3: h2 = A @ h1s; stage 4: y = h2*s4 + bias
                for og in range(nO // MG):
                    pso = [pspool.tile([P, TC], F32, tag="ps", name="ps")
                           for _ in range(MG)]
                    for m in range(nM):
                        wt2 = wpool.tile([P, MG * P], BF16, tag="wa")
                        nc.gpsimd.dma_start(
                            wt2[:], ATd[m * P:(m + 1) * P,
                                        og * MG * P:(og + 1) * MG * P])
                        for o_ in range(MG):
                            lhsT = wt2[:, o_ * P:(o_ + 1) * P]
                            nc.tensor.matmul(pso[o_][:], lhsT, h1[m][:],
                                             start=(m == 0), stop=(m == nM - 1))
                    for o_ in range(MG):
                        o = og * MG + o_
                        yt = ypool.tile([P, TC], F32, tag="yt")
                        nc.vector.tensor_scalar(
                            yt[:], pso[o_][:], s4_t[:, o:o + 1], bi_t[:, o:o + 1],
                            mult, add)
                        nc.scalar.dma_start(yT[o * P:(o + 1) * P, t0:t0 + TC], yt[:])

    nc.compile()
    return nc


_NC_CACHE = None


def _get_nc():
    global _NC_CACHE
    if _NC_CACHE is None:
        _NC_CACHE = _build_nc()
    return _NC_CACHE


def _col_major(v):
    return np.ascontiguousarray(
        np.asarray(v, dtype=np.float32).reshape(-1, P).T)


def make_in_maps(x, scaling0, B, scaling2, A, scaling4, bias):
    x = np.asarray(x, dtype=np.float32)
    xf = np.ascontiguousarray(x.reshape(BATCH * SEQ, IN_D))
    BT = np.ascontiguousarray(np.asarray(B, dtype=np.float32).T
                              ).astype(ml_dtypes.bfloat16)
    AT = np.ascontiguousarray(np.asarray(A, dtype=np.float32).T
                              ).astype(ml_dtypes.bfloat16)
    sc = np.ascontiguousarray(np.concatenate(
        [_col_major(v) for v in (scaling0, scaling2, scaling4, bias)], axis=1))

    in_maps = []
    for c in range(N_CORES):
        xs = xf[c * T_CORE:(c + 1) * T_CORE]
        in_maps.append({
            "xT": np.ascontiguousarray(xs.T),
            "BT": BT, "AT": AT, "sc": sc,
        })
    return in_maps


def kernel(x, scaling0, B, scaling2, A, scaling4, bias):
    # The profile hook isn't available in every environment; force the
    # plain execution path.
    os.environ.setdefault("BASS_NEVER_TRACE", "1")

    in_maps = make_in_maps(x, scaling0, B, scaling2, A, scaling4, bias)
    nc = _get_nc()
    res = bass_utils.run_bass_kernel_spmd(
        nc, in_maps, core_ids=list(range(N_CORES)))

    y = np.empty((BATCH * SEQ, OUT_D), dtype=np.float32)
    for c in range(N_CORES):
        y[c * T_CORE:(c + 1) * T_CORE] = res.results[c]["yT"].T
    return y.reshape(BATCH, SEQ, OUT_D)


# revision 6
# speedup vs baseline: 1.1300x; 1.0004x over previous
"""Trainium2 Bass kernel for DoubleBinaryLinear:
    y = ((x * s0) @ B.T * s2) @ A.T * s4 + bias
with x [4, 2048, 4096] fp32 and binary (+-1) B, A [4096, 4096].

Strategy
--------
Data-parallel over tokens: the 8192 tokens are split 1024 per NeuronCore
(8 cores), each core runs the full two-layer pipeline on its token slice.
No collectives. Weights are replicated (B.T / A.T, cast to bf16 — exact
for +-1 entries).

On-chip dataflow keeps activations transposed ([feature_partition,
token_free]) so every diagonal scaling is a per-partition tensor_scalar op
and matmul contractions run over the partition axis:

  stage 0: z  = bf16(xT * s0)
  stage 1: h1 = B @ z       -- PSUM fp32 accumulation
  stage 2: h1s = bf16(h1 * s2)
  stage 3: h2 = A @ h1s     -- PSUM fp32 accumulation
  stage 4: yT = h2 * s4 + bias

Single-pass bf16: activations are rounded to bf16 once per stage while
the +-1 weights are exact in bf16, so each matmul runs one 1-cycle/row
bf16 pass (216 ns steady issue for N=512; fp8 DoubleRow measures the
same 216 ns/instr on HW, so fp8 hi/lo packing buys no wall-clock).
End-to-end error ~2.5e-3 of the output scale vs the 2e-2 gate.

TensorE bound: 4096 matmuls x 216 ns = 885 us/core.  Overhead trims:
 - B weights stream on the Sync DMA queue, A weights + x tiles on the
   GpSimd queue (decouples x-DMA issue from the dependent scale ops,
   which head-of-line blocked the Scalar queue).
 - stage-0 scaling alternates Scalar/Vector engines so z production
   (~400 ns/tile) outruns matmul consumption (853 ns/tile).
 - a burst of dummy matmuls right after the preamble starts the PE
   DVFS ramp (~11 us low-clock) before real data arrives.
 - the final output group runs its four 32-matmul PSUM chains
   back-to-back (weights preloaded) so evictions + y stores overlap
   the tail instead of serializing after the last matmul.
"""

import os

import numpy as np
import ml_dtypes

import concourse.bacc as bacc
import concourse.mybir as mybir
from concourse import tile
from concourse import bass_utils

P = 128
F32 = mybir.dt.float32
BF16 = mybir.dt.bfloat16

IN_D = 4096
MID_D = 4096
OUT_D = 4096
BATCH = 4
SEQ = 2048
N_CORES = 8
T_CORE = (BATCH * SEQ) // N_CORES   # 1024 tokens per core
TC = 512                            # token chunk = matmul moving free dim
MG = 4                              # m-tiles per PSUM group
N_WARM = 10                         # dummy matmuls to start the DVFS ramp


def _build_nc():
    nI, nM, nO, nC = IN_D // P, MID_D // P, OUT_D // P, T_CORE // TC

    nc = bacc.Bacc(None, target_bir_lowering=False)
    xT = nc.dram_tensor("xT", [IN_D, T_CORE], F32, kind="ExternalInput")
    BTd = nc.dram_tensor("BT", [IN_D, MID_D], BF16, kind="ExternalInput")
    ATd = nc.dram_tensor("AT", [MID_D, OUT_D], BF16, kind="ExternalInput")
    nSC = (IN_D + MID_D + 2 * OUT_D) // P
    scd = nc.dram_tensor("sc", [P, nSC], F32, kind="ExternalInput")
    yT = nc.dram_tensor("yT", [OUT_D, T_CORE], F32, kind="ExternalOutput")

    mult = mybir.AluOpType.mult
    add = mybir.AluOpType.add

    with tile.TileContext(nc) as tc:
        with (
            tc.tile_pool(name="consts", bufs=1) as cpool,
            tc.tile_pool(name="zbuf", bufs=1) as zpool,
            tc.tile_pool(name="h1buf", bufs=1) as hpool,
            tc.tile_pool(name="xin", bufs=12) as xpool,
            tc.tile_pool(name="wts", bufs=16) as wpool,
            tc.tile_pool(name="wlast", bufs=1) as wlpool,
            tc.tile_pool(name="yout", bufs=3) as ypool,
            tc.tile_pool(name="psum", bufs=7, space="PSUM") as pspool,
            tc.tile_pool(name="psdum", bufs=1, space="PSUM") as dumpool,
        ):
            sc_t = cpool.tile([P, nSC], F32, tag="sc")
            nc.sync.dma_start(sc_t[:], scd[:, :])
            s0_t = sc_t[:, 0:nI]
            s2_t = sc_t[:, nI:nI + nM]
            s4_t = sc_t[:, nI + nM:nI + nM + nO]
            bi_t = sc_t[:, nI + nM + nO:nSC]

            # PE warmup: zero tiles + a burst of dummy matmuls to start
            # the DVFS ramp while the first real data is still in flight.
            wdum = cpool.tile([P, P], BF16, tag="wdum")
            zdum = cpool.tile([P, TC], BF16, tag="zdum")
            nc.vector.memset(wdum[:], 0)
            nc.vector.memset(zdum[:], 0)
            psd = dumpool.tile([P, TC], F32, tag="psd")
            for _ in range(N_WARM):
                nc.tensor.matmul(psd[:], wdum[:], zdum[:], start=True,
                                 stop=True)

            for c in range(nC):
                t0 = c * TC
                # stage 0: load x, scale by s0, round to bf16
                z = [zpool.tile([P, TC], BF16, tag=f"z{i}", name=f"z{i}")
                     for i in range(nI)]
                for i in range(nI):
                    xt = xpool.tile([P, TC], F32, tag="xt")
                    nc.gpsimd.dma_start(xt[:], xT[i * P:(i + 1) * P, t0:t0 + TC])
                    # alternate engines so z production (~400 ns/tile)
                    # outruns matmul consumption (853 ns/tile)
                    if (c == 0 and i < 2) or i % 2 == 1:
                        nc.vector.tensor_scalar_mul(
                            z[i][:], xt[:], s0_t[:, i:i + 1])
                    else:
                        nc.scalar.activation(z[i][:], xt[:],
                                             mybir.ActivationFunctionType.Copy,
                                             scale=s0_t[:, i:i + 1])

                if c == nC - 1:
                    # preload the last output group's A panel so its four
                    # PSUM chains can run back-to-back at the very end
                    wl = [wlpool.tile([P, MG * P], BF16, tag=f"wl{m}",
                                      name=f"wl{m}")
                          for m in range(nM)]
                    for m in range(nM):
                        nc.gpsimd.dma_start(
                            wl[m][:], ATd[m * P:(m + 1) * P,
                                          (nO - MG) * P:nO * P])

                # stage 1: h1 = B @ z; stage 2: h1s = bf16(h1 * s2)
                h1 = [hpool.tile([P, TC], BF16, tag=f"h1{m}", name=f"h1{m}")
                      for m in range(nM)]
                for mg in range(nM // MG):
                    pss = [pspool.tile([P, TC], F32, tag="ps", name="ps")
                           for _ in range(MG)]
                    for i in range(nI):
                        wt = wpool.tile([P, MG * P], BF16, tag="wb")
                        nc.sync.dma_start(
                            wt[:], BTd[i * P:(i + 1) * P,
                                       mg * MG * P:(mg + 1) * MG * P])
                        for m_ in range(MG):
                            lhsT = wt[:, m_ * P:(m_ + 1) * P]
                            nc.tensor.matmul(pss[m_][:], lhsT, z[i][:],
                                             start=(i == 0), stop=(i == nI - 1))
                    for m_ in range(MG):
                        m = mg * MG + m_
                        nc.scalar.activation(
                            h1[m][:], pss[m_][:],
                            mybir.ActivationFunctionType.Copy,
                            scale=s2_t[:, m:m + 1])

                # stage 3: h2 = A @ h1s; stage 4: y = h2*s4 + bias
                for og in range(nO // MG):
                    last_group = c == nC - 1 and og == nO // MG - 1
                    pso = [pspool.tile([P, TC], F32, tag="ps", name="ps")
                           for _ in range(MG)]
                    if last_group:
                        # sequential chains per output tile; weights were
                        # preloaded, so evictions overlap the remaining
                        # chains instead of serializing after the end
                        for o_ in range(MG):
                            for m in range(nM):
                                lhsT = wl[m][:, o_ * P:(o_ + 1) * P]
                                nc.tensor.matmul(pso[o_][:], lhsT, h1[m][:],
                                                 start=(m == 0),
                                                 stop=(m == nM - 1))
                            o = og * MG + o_
                            yt = ypool.tile([P, TC], F32, tag="yt")
                            nc.vector.tensor_scalar(
                                yt[:], pso[o_][:], s4_t[:, o:o + 1],
                                bi_t[:, o:o + 1], mult, add)
                            nc.scalar.dma_start(
                                yT[o * P:(o + 1) * P, t0:t0 + TC], yt[:])
                        continue
                    for m in range(nM):
                        wt2 = wpool.tile([P, MG * P], BF16, tag="wa")
                        nc.gpsimd.dma_start(
                            wt2[:], ATd[m * P:(m + 1) * P,
                                        og * MG * P:(og + 1) * MG * P])
                        for o_ in range(MG):
                            lhsT = wt2[:, o_ * P:(o_ + 1) * P]
                            nc.tensor.matmul(pso[o_][:], lhsT, h1[m][:],
                                             start=(m == 0), stop=(m == nM - 1))
                    for o_ in range(MG):
                        o = og * MG + o_
                        yt = ypool.tile([P, TC], F32, tag="yt")
                        nc.vector.tensor_scalar(
                            yt[:], pso[o_][:], s4_t[:, o:o + 1], bi_t[:, o:o + 1],
                            mult, add)
                        nc.scalar.dma_start(yT[o * P:(o + 1) * P, t0:t0 + TC], yt[:])

    nc.compile()
    return nc


_NC_CACHE = None


def _get_nc():
    global _NC_CACHE
    if _NC_CACHE is None:
        _NC_CACHE = _build_nc()
    return _NC_CACHE


def _col_major(v):
    return np.ascontiguousarray(
        np.asarray(v, dtype=np.float32).reshape(-1, P).T)


def make_in_maps(x, scaling0, B, scaling2, A, scaling4, bias):
    x = np.asarray(x, dtype=np.float32)
    xf = np.ascontiguousarray(x.reshape(BATCH * SEQ, IN_D))
    BT = np.ascontiguousarray(np.asarray(B, dtype=np.float32).T
                              ).astype(ml_dtypes.bfloat16)
    AT = np.ascontiguousarray(np.asarray(A, dtype=np.float32).T
                              ).astype(ml_dtypes.bfloat16)
    sc = np.ascontiguousarray(np.concatenate(
        [_col_major(v) for v in (scaling0, scaling2, scaling4, bias)], axis=1))

    in_maps = []
    for c in range(N_CORES):
        xs = xf[c * T_CORE:(c + 1) * T_CORE]
        in_maps.append({
            "xT": np.ascontiguousarray(xs.T),
            "BT": BT, "AT": AT, "sc": sc,
        })
    return in_maps


def kernel(x, scaling0, B, scaling2, A, scaling4, bias):
    # The profile hook isn't available in every environment; force the
    # plain execution path.
    os.environ.setdefault("BASS_NEVER_TRACE", "1")

    in_maps = make_in_maps(x, scaling0, B, scaling2, A, scaling4, bias)
    nc = _get_nc()
    res = bass_utils.run_bass_kernel_spmd(
        nc, in_maps, core_ids=list(range(N_CORES)))

    y = np.empty((BATCH * SEQ, OUT_D), dtype=np.float32)
    for c in range(N_CORES):
        y[c * T_CORE:(c + 1) * T_CORE] = res.results[c]["yT"].T
    return y.reshape(BATCH, SEQ, OUT_D)


# revision 7
# speedup vs baseline: 1.1306x; 1.0006x over previous
"""Trainium2 Bass kernel for DoubleBinaryLinear:
    y = ((x * s0) @ B.T * s2) @ A.T * s4 + bias
with x [4, 2048, 4096] fp32 and binary (+-1) B, A [4096, 4096].

Strategy
--------
Data-parallel over tokens: the 8192 tokens are split 1024 per NeuronCore
(8 cores), each core runs the full two-layer pipeline on its token slice.
No collectives. Weights are replicated (B.T / A.T, cast to bf16 — exact
for +-1 entries).

On-chip dataflow keeps activations transposed ([feature_partition,
token_free]) so every diagonal scaling is a per-partition tensor_scalar op
and matmul contractions run over the partition axis:

  stage 0: z  = bf16(xT * s0)
  stage 1: h1 = B @ z       -- PSUM fp32 accumulation
  stage 2: h1s = bf16(h1 * s2)
  stage 3: h2 = A @ h1s     -- PSUM fp32 accumulation
  stage 4: yT = h2 * s4 + bias

Single-pass bf16: activations are rounded to bf16 once per stage while
the +-1 weights are exact in bf16, so each matmul runs one 1-cycle/row
bf16 pass (216 ns steady issue for N=512; fp8 DoubleRow measures the
same 216 ns/instr on HW, so fp8 hi/lo packing buys no wall-clock).
End-to-end error ~2.5e-3 of the output scale vs the 2e-2 gate.

TensorE bound: 4096 matmuls x 216 ns = 885 us/core.  Overhead trims:
 - B weights stream on the Sync DMA queue, A weights + x tiles on the
   GpSimd queue (decouples x-DMA issue from the dependent scale ops,
   which head-of-line blocked the Scalar queue).
 - stage-0 scaling alternates Scalar/Vector engines so z production
   (~400 ns/tile) outruns matmul consumption (853 ns/tile).
 - a burst of dummy matmuls right after the preamble starts the PE
   DVFS ramp (~11 us low-clock) before real data arrives.
 - the final output group runs its four 32-matmul PSUM chains
   back-to-back (weights preloaded) so evictions + y stores overlap
   the tail instead of serializing after the last matmul.
"""

import os

import numpy as np
import ml_dtypes

import concourse.bacc as bacc
import concourse.mybir as mybir
from concourse import tile
from concourse import bass_utils

P = 128
F32 = mybir.dt.float32
BF16 = mybir.dt.bfloat16

IN_D = 4096
MID_D = 4096
OUT_D = 4096
BATCH = 4
SEQ = 2048
N_CORES = 8
T_CORE = (BATCH * SEQ) // N_CORES   # 1024 tokens per core
TC = 512                            # token chunk = matmul moving free dim
MG = 4                              # m-tiles per PSUM group
N_WARM = 24                         # dummy matmuls to start the DVFS ramp


def _build_nc():
    nI, nM, nO, nC = IN_D // P, MID_D // P, OUT_D // P, T_CORE // TC

    nc = bacc.Bacc(None, target_bir_lowering=False)
    xT = nc.dram_tensor("xT", [IN_D, T_CORE], F32, kind="ExternalInput")
    BTd = nc.dram_tensor("BT", [IN_D, MID_D], BF16, kind="ExternalInput")
    ATd = nc.dram_tensor("AT", [MID_D, OUT_D], BF16, kind="ExternalInput")
    nSC = (IN_D + MID_D + 2 * OUT_D) // P
    scd = nc.dram_tensor("sc", [P, nSC], F32, kind="ExternalInput")
    yT = nc.dram_tensor("yT", [OUT_D, T_CORE], F32, kind="ExternalOutput")

    mult = mybir.AluOpType.mult
    add = mybir.AluOpType.add

    with tile.TileContext(nc) as tc:
        with (
            tc.tile_pool(name="consts", bufs=1) as cpool,
            tc.tile_pool(name="zbuf", bufs=1) as zpool,
            tc.tile_pool(name="h1buf", bufs=1) as hpool,
            tc.tile_pool(name="xin", bufs=12) as xpool,
            tc.tile_pool(name="wts", bufs=16) as wpool,
            tc.tile_pool(name="wlast", bufs=1) as wlpool,
            tc.tile_pool(name="yout", bufs=3) as ypool,
            tc.tile_pool(name="psum", bufs=7, space="PSUM") as pspool,
            tc.tile_pool(name="psdum", bufs=1, space="PSUM") as dumpool,
        ):
            sc_t = cpool.tile([P, nSC], F32, tag="sc")
            nc.sync.dma_start(sc_t[:], scd[:, :])
            s0_t = sc_t[:, 0:nI]
            s2_t = sc_t[:, nI:nI + nM]
            s4_t = sc_t[:, nI + nM:nI + nM + nO]
            bi_t = sc_t[:, nI + nM + nO:nSC]

            # PE warmup: zero tiles + a burst of dummy matmuls to start
            # the DVFS ramp while the first real data is still in flight.
            wdum = cpool.tile([P, P], BF16, tag="wdum")
            zdum = cpool.tile([P, TC], BF16, tag="zdum")
            nc.vector.memset(wdum[:], 0)
            nc.vector.memset(zdum[:], 0)
            psd = dumpool.tile([P, TC], F32, tag="psd")
            for _ in range(N_WARM):
                nc.tensor.matmul(psd[:], wdum[:], zdum[:], start=True,
                                 stop=True)

            for c in range(nC):
                t0 = c * TC
                # stage 0: load x, scale by s0, round to bf16
                z = [zpool.tile([P, TC], BF16, tag=f"z{i}", name=f"z{i}")
                     for i in range(nI)]
                for i in range(nI):
                    xt = xpool.tile([P, TC], F32, tag="xt")
                    nc.gpsimd.dma_start(xt[:], xT[i * P:(i + 1) * P, t0:t0 + TC])
                    # alternate engines so z production (~400 ns/tile)
                    # outruns matmul consumption (853 ns/tile)
                    if (c == 0 and i < 2) or i % 2 == 1:
                        nc.vector.tensor_scalar_mul(
                            z[i][:], xt[:], s0_t[:, i:i + 1])
                    else:
                        nc.scalar.activation(z[i][:], xt[:],
                                             mybir.ActivationFunctionType.Copy,
                                             scale=s0_t[:, i:i + 1])

                if c == nC - 1:
                    # preload the last output group's A panel so its four
                    # PSUM chains can run back-to-back at the very end
                    wl = [wlpool.tile([P, MG * P], BF16, tag=f"wl{m}",
                                      name=f"wl{m}")
                          for m in range(nM)]
                    for m in range(nM):
                        nc.gpsimd.dma_start(
                            wl[m][:], ATd[m * P:(m + 1) * P,
                                          (nO - MG) * P:nO * P])

                # stage 1: h1 = B @ z; stage 2: h1s = bf16(h1 * s2)
                h1 = [hpool.tile([P, TC], BF16, tag=f"h1{m}", name=f"h1{m}")
                      for m in range(nM)]
                for mg in range(nM // MG):
                    pss = [pspool.tile([P, TC], F32, tag="ps", name="ps")
                           for _ in range(MG)]
                    for i in range(nI):
                        wt = wpool.tile([P, MG * P], BF16, tag="wb")
                        nc.sync.dma_start(
                            wt[:], BTd[i * P:(i + 1) * P,
                                       mg * MG * P:(mg + 1) * MG * P])
                        for m_ in range(MG):
                            lhsT = wt[:, m_ * P:(m_ + 1) * P]
                            nc.tensor.matmul(pss[m_][:], lhsT, z[i][:],
                                             start=(i == 0), stop=(i == nI - 1))
                    for m_ in range(MG):
                        m = mg * MG + m_
                        nc.scalar.activation(
                            h1[m][:], pss[m_][:],
                            mybir.ActivationFunctionType.Copy,
                            scale=s2_t[:, m:m + 1])

                # stage 3: h2 = A @ h1s; stage 4: y = h2*s4 + bias
                for og in range(nO // MG):
                    last_group = c == nC - 1 and og == nO // MG - 1
                    pso = [pspool.tile([P, TC], F32, tag="ps", name="ps")
                           for _ in range(MG)]
                    if last_group:
                        # sequential chains per output tile; weights were
                        # preloaded, so evictions overlap the remaining
                        # chains instead of serializing after the end
                        for o_ in range(MG):
                            for m in range(nM):
                                lhsT = wl[m][:, o_ * P:(o_ + 1) * P]
                                nc.tensor.matmul(pso[o_][:], lhsT, h1[m][:],
                                                 start=(m == 0),
                                                 stop=(m == nM - 1))
                            o = og * MG + o_
                            yt = ypool.tile([P, TC], F32, tag="yt")
                            nc.vector.tensor_scalar(
                                yt[:], pso[o_][:], s4_t[:, o:o + 1],
                                bi_t[:, o:o + 1], mult, add)
                            nc.scalar.dma_start(
                                yT[o * P:(o + 1) * P, t0:t0 + TC], yt[:])
                        continue
                    for m in range(nM):
                        wt2 = wpool.tile([P, MG * P], BF16, tag="wa")
                        nc.gpsimd.dma_start(
                            wt2[:], ATd[m * P:(m + 1) * P,
                                        og * MG * P:(og + 1) * MG * P])
                        for o_ in range(MG):
                            lhsT = wt2[:, o_ * P:(o_ + 1) * P]
                            nc.tensor.matmul(pso[o_][:], lhsT, h1[m][:],
                                             start=(m == 0), stop=(m == nM - 1))
                    for o_ in range(MG):
                        o = og * MG + o_
                        yt = ypool.tile([P, TC], F32, tag="yt")
                        nc.vector.tensor_scalar(
                            yt[:], pso[o_][:], s4_t[:, o:o + 1], bi_t[:, o:o + 1],
                            mult, add)
                        nc.scalar.dma_start(yT[o * P:(o + 1) * P, t0:t0 + TC], yt[:])

    nc.compile()
    return nc


_NC_CACHE = None


def _get_nc():
    global _NC_CACHE
    if _NC_CACHE is None:
        _NC_CACHE = _build_nc()
    return _NC_CACHE


def _col_major(v):
    return np.ascontiguousarray(
        np.asarray(v, dtype=np.float32).reshape(-1, P).T)


def make_in_maps(x, scaling0, B, scaling2, A, scaling4, bias):
    x = np.asarray(x, dtype=np.float32)
    xf = np.ascontiguousarray(x.reshape(BATCH * SEQ, IN_D))
    BT = np.ascontiguousarray(np.asarray(B, dtype=np.float32).T
                              ).astype(ml_dtypes.bfloat16)
    AT = np.ascontiguousarray(np.asarray(A, dtype=np.float32).T
                              ).astype(ml_dtypes.bfloat16)
    sc = np.ascontiguousarray(np.concatenate(
        [_col_major(v) for v in (scaling0, scaling2, scaling4, bias)], axis=1))

    in_maps = []
    for c in range(N_CORES):
        xs = xf[c * T_CORE:(c + 1) * T_CORE]
        in_maps.append({
            "xT": np.ascontiguousarray(xs.T),
            "BT": BT, "AT": AT, "sc": sc,
        })
    return in_maps


def kernel(x, scaling0, B, scaling2, A, scaling4, bias):
    # The profile hook isn't available in every environment; force the
    # plain execution path.
    os.environ.setdefault("BASS_NEVER_TRACE", "1")

    in_maps = make_in_maps(x, scaling0, B, scaling2, A, scaling4, bias)
    nc = _get_nc()
    res = bass_utils.run_bass_kernel_spmd(
        nc, in_maps, core_ids=list(range(N_CORES)))

    y = np.empty((BATCH * SEQ, OUT_D), dtype=np.float32)
    for c in range(N_CORES):
        y[c * T_CORE:(c + 1) * T_CORE] = res.results[c]["yT"].T
    return y.reshape(BATCH, SEQ, OUT_D)
